# revision 3
# baseline (speedup 1.0000x reference)
"""Trainium2 Bass kernel for the bidirectional GRU-ODE (nn_CODEBiGRU).

All-TensorEngine design (collective-free, 2 cores, one chain per core):
  - Host folds G = W1 @ W2, g0 = W1 @ b2.  RK4 runs in pre-activation
    space: d_q = G t_q, u_{q+1} = u1 + c_q d_q; h-updates telescope into
    h_T = (h0 + 15 dt b2) + dt/6 W2 (sum_s T_s).
  - Everything is column-tiled: cols[p, j] = flat[32p + j].  All matvecs
    use stationary-weight matmuls: lhsT(k,m)[p,q] = W[32q+m, 32p+k],
    out cols[q, m] accumulated over k in PSUM, m-outer/k-inner
    (sequential accumulation groups; interleaved groups are numerically
    broken on this HW).
  - The 61 sequential G-matvecs use SBUF-resident fp8-e3m4 G (scaled
    x64, t scaled x8, 1/512 folded into step constants): 1024 matmuls /
    37us per eval.  Measured end-to-end rel-err ~9e-3 (gate 2e-2).
  - Init + finale (W1, W2, Wx, Wh x2, h2o-half) stream bf16 weights in
    the same layout through double-buffered 4MB chunks on both DGE
    queues (SP + Activation), 1024 matmuls each, 8 PSUM partial columns
    reduced on DVE.  The i2h x-part is computed once and reused for both
    GRU gates.
  - Whole iteration sits in For_i(0, niters) for delta-timing.
  - Host: weight re-layouts cached by fingerprint; per-core weight
    tensors are device_put once and reused across calls (axon tunnel is
    ~40MB/s, so re-upload would dominate).
"""
import sys
import numpy as np

sys.path.insert(0, "/opt/trn_rl_repo")

import ml_dtypes  # noqa: E402
import jax  # noqa: E402
import concourse.bass as bass  # noqa: E402
import concourse.tile as tile  # noqa: E402
from concourse import bacc, mybir, bass2jax  # noqa: E402

NH = 4096
NB = 32                 # col blocks (tiled [128, 32], flat = 32p + b)
NSTEP = 15
F32 = mybir.dt.float32
BF16 = mybir.dt.bfloat16
F8E3 = mybir.dt.float8e3
AF = mybir.ActivationFunctionType
ALU = mybir.AluOpType

SM = 64.0               # fp8 scale on G
STQ = 8.0               # fp8 scale on t
SMT = SM * STQ

KBSZ = 2                # streamed slab-group size (k-tiles per chunk)
NKB = NB // KBSZ        # chunks per streamed matvec
NREG = 4                # stream regions (DMA lookahead = NREG - 1 chunks)

C_B1, C_CG0A, C_CG0B, C_H0SD, C_BG = range(5)
NCST = 5
S_CA2, S_CB2, S_CD6, S_CD = range(4)


def _build(niters=1):
    nc = bacc.Bacc("TRN2", target_bir_lowering=False, debug=False,
                   num_devices=2)

    mq_d = nc.dram_tensor("mq", [128, NB * NB * 128], F8E3, kind="ExternalInput")
    w1_d = nc.dram_tensor("w1", [128, NB * NB * 128], BF16, kind="ExternalInput")
    w2_d = nc.dram_tensor("w2", [128, NB * NB * 128], BF16, kind="ExternalInput")
    wx_d = nc.dram_tensor("wx", [128, NB * NB * 128], BF16, kind="ExternalInput")
    wh_d = nc.dram_tensor("wh", [128, NB * NB * 128], BF16, kind="ExternalInput")
    ho_d = nc.dram_tensor("ho", [128, NB * NB * 128], BF16, kind="ExternalInput")
    h0c_d = nc.dram_tensor("h0c", [128, NB], BF16, kind="ExternalInput")
    xc_d = nc.dram_tensor("xc", [128, NB], BF16, kind="ExternalInput")
    cst_d = nc.dram_tensor("csts", [128, NB * NCST], F32, kind="ExternalInput")
    scal_d = nc.dram_tensor("scal", [128, 4], F32, kind="ExternalInput")

    o_d = nc.dram_tensor("o_part", [128, NB], F32, kind="ExternalOutput")
    hn_d = nc.dram_tensor("hn", [128, NB], F32, kind="ExternalOutput")

    def aview(d):
        return d[:].rearrange("p (k m q) -> p k m q", k=NB, m=NB)

    mqv, w1v, w2v, wxv, whv, hov = map(
        aview, (mq_d, w1_d, w2_d, wx_d, wh_d, ho_d))

    with tile.TileContext(nc) as tc:
        with tc.tile_pool(name="base", bufs=1) as base, \
             tc.tile_pool(name="psum", bufs=1, space="PSUM") as pp:

            ma = base.tile([128, NB, NB, 128], F8E3, tag="ma")       # 128KB/p
            sregs = [base.tile([128, KBSZ, NB, 128], BF16,
                               tag=f"sreg{i}", name=f"sreg{i}")
                     for i in range(NREG)]                           # 16KB/p ea

            u1 = base.tile([128, NB], F32, tag="u1")
            ux = base.tile([128, NB], F32, tag="ux")
            sA = base.tile([128, NB], F32, tag="sA")
            sB = base.tile([128, NB], F32, tag="sB")
            dsum = base.tile([128, NB], F32, tag="dsum")
            tsum = base.tile([128, NB], F32, tag="tsum")
            tbf = base.tile([128, NB], BF16, tag="tbf")
            tq = base.tile([128, NB], F8E3, tag="tq")
            tt = base.tile([128, NB], BF16, tag="tt")
            h0c = base.tile([128, NB], BF16, tag="h0c")
            xc = base.tile([128, NB], BF16, tag="xc")
            px = base.tile([128, NB], F32, tag="px")
            tmp = base.tile([128, NB], F32, tag="tmp")
            tmp2 = base.tile([128, NB], F32, tag="tmp2")
            hfin = base.tile([128, NB], F32, tag="hfin")
            gg = base.tile([128, NB], F32, tag="gg")
            csts = base.tile([128, NB, NCST], F32, tag="csts")
            scal = base.tile([128, 4], F32, tag="scal")
            ps = pp.tile([128, NB], F32, tag="ps")
            ps2 = pp.tile([128, NB, NKB], F32, tag="ps2")

            qeng = [nc.sync, nc.scalar]

            def pe_mv(wview, rhs_cols, acc):
                """acc[q, m] = sum_k W(k,m).T @ rhs[:, k], streamed bf16
                weights, NKB psum partials reduced on DVE."""
                def ld(kb, qi):
                    qeng[qi % 2].dma_start(
                        sregs[kb % NREG][:].rearrange("p a m q -> p (a m q)"),
                        wview[:, kb * KBSZ:(kb + 1) * KBSZ, :, :].rearrange(
                            "p a m q -> p (a m q)"))
                for j in range(min(NREG - 1, NKB)):
                    ld(j, j)
                for kb in range(NKB):
                    if kb + NREG - 1 < NKB:
                        ld(kb + NREG - 1, kb + NREG - 1)
                    reg = sregs[kb % NREG]
                    for m in range(NB):
                        for kk in range(KBSZ):
                            nc.tensor.matmul(
                                ps2[:, m, kb:kb + 1],
                                reg[:, kk, m, :],
                                rhs_cols[:, kb * KBSZ + kk:kb * KBSZ + kk + 1],
                                start=(kk == 0), stop=(kk == KBSZ - 1))
                nc.vector.tensor_reduce(acc[:], ps2[:], mybir.AxisListType.X,
                                        ALU.add)

            def mm_eval():
                """ps[:, m] = sum_k MA(k,m).T @ tq[:, k]  (= SMT * G @ t)."""
                for m in range(NB):
                    for k in range(NB):
                        nc.tensor.matmul(
                            ps[:, m:m + 1],
                            ma[:, k, m, :],
                            tq[:, k:k + 1],
                            start=(k == 0), stop=(k == NB - 1))

            def stage_tail(u_next_like):
                nc.scalar.activation(tbf[:], u_next_like[:], AF.Tanh)
                nc.vector.tensor_scalar_mul(tq[:], tbf[:], STQ)

            with tc.For_i(0, niters) as _it:
                nc.sync.dma_start(csts[:].rearrange("p a c -> p (a c)"),
                                  cst_d[:])
                nc.sync.dma_start(scal[:], scal_d[:])
                nc.sync.dma_start(h0c[:], h0c_d[:])
                nc.sync.dma_start(xc[:], xc_d[:])
                capA2 = scal[:, S_CA2:S_CA2 + 1]
                capB2 = scal[:, S_CB2:S_CB2 + 1]
                capD6 = scal[:, S_CD6:S_CD6 + 1]
                capD = scal[:, S_CD:S_CD + 1]

                # ---- u1 = W1 @ h0 + b1 (streamed PE matvec) ----
                pe_mv(w1v, h0c, u1)
                nc.vector.tensor_add(u1[:], u1[:], csts[:, :, C_B1])

                # load G (fp8, 16MB) in 8 chunks, alternating DGE queues
                for i in range(8):
                    qeng[i % 2].dma_start(
                        ma[:, i * 4:(i + 1) * 4, :, :].rearrange(
                            "p a m q -> p (a m q)"),
                        mqv[:, i * 4:(i + 1) * 4, :, :].rearrange(
                            "p a m q -> p (a m q)"))

                nc.scalar.activation(tbf[:], u1[:], AF.Tanh)
                nc.vector.tensor_copy(tsum[:], tbf[:])
                nc.vector.tensor_scalar_mul(tq[:], tbf[:], STQ)

                # ---- RK4: 15 steps x 4 PE-matvecs ----
                with tc.For_i(0, NSTEP) as _s:
                    nc.vector.tensor_add(sA[:], u1[:], csts[:, :, C_CG0A])
                    nc.vector.tensor_add(sB[:], u1[:], csts[:, :, C_CG0B])
                    # q1
                    mm_eval()
                    nc.vector.tensor_scalar_mul(dsum[:], ps[:], 1.0)
                    nc.vector.scalar_tensor_tensor(
                        ux[:], ps[:], capA2, sA[:], ALU.mult, ALU.add)
                    stage_tail(ux)
                    nc.vector.scalar_tensor_tensor(
                        tsum[:], tbf[:], 2.0, tsum[:], ALU.mult, ALU.add)
                    # q2
                    mm_eval()
                    nc.vector.scalar_tensor_tensor(
                        dsum[:], ps[:], 2.0, dsum[:], ALU.mult, ALU.add)
                    nc.vector.scalar_tensor_tensor(
                        ux[:], ps[:], capA2, sA[:], ALU.mult, ALU.add)
                    stage_tail(ux)
                    nc.vector.scalar_tensor_tensor(
                        tsum[:], tbf[:], 2.0, tsum[:], ALU.mult, ALU.add)
                    # q3
                    mm_eval()
                    nc.vector.scalar_tensor_tensor(
                        dsum[:], ps[:], 2.0, dsum[:], ALU.mult, ALU.add)
                    nc.vector.scalar_tensor_tensor(
                        ux[:], ps[:], capB2, sB[:], ALU.mult, ALU.add)
                    stage_tail(ux)
                    nc.vector.tensor_add(tsum[:], tsum[:], tbf[:])
                    # q4
                    mm_eval()
                    nc.vector.tensor_add(dsum[:], dsum[:], ps[:])
                    nc.vector.scalar_tensor_tensor(
                        u1[:], dsum[:], capD6, u1[:], ALU.mult, ALU.add)
                    nc.vector.tensor_add(u1[:], u1[:], csts[:, :, C_CG0B])
                    stage_tail(u1)
                    nc.vector.tensor_add(tsum[:], tsum[:], tbf[:])

                # tsum overcounts tanh(u1_15): subtract
                nc.vector.tensor_sub(tsum[:], tsum[:], tbf[:])

                if True:
                    # ---- h_T = (h0 + 15 dt b2) + dt/6 W2 tsum ----
                    nc.vector.tensor_copy(tt[:], tsum[:])
                    pe_mv(w2v, tt, tmp)
                    nc.vector.scalar_tensor_tensor(
                        hfin[:], tmp[:], capD, csts[:, :, C_H0SD],
                        ALU.mult, ALU.add)

                    # ---- GRU (chain-local, all in col space) ----
                    pe_mv(wxv, xc, px)
                    nc.vector.tensor_add(px[:], px[:], csts[:, :, C_BG])
                    nc.vector.tensor_copy(tt[:], hfin[:])
                    pe_mv(whv, tt, tmp)
                    nc.vector.tensor_add(gg[:], px[:], tmp[:])
                    nc.scalar.activation(gg[:], gg[:], AF.Sigmoid)
                    nc.vector.tensor_mul(tt[:], gg[:], hfin[:])
                    pe_mv(whv, tt, tmp)
                    nc.vector.tensor_add(tmp[:], px[:], tmp[:])
                    nc.scalar.activation(tmp[:], tmp[:], AF.Tanh)
                    # h_new = h_hat + g*(h - h_hat)
                    nc.vector.tensor_sub(tmp2[:], hfin[:], tmp[:])
                    nc.vector.tensor_mul(tmp2[:], gg[:], tmp2[:])
                    nc.vector.tensor_add(tmp[:], tmp[:], tmp2[:])
                    nc.sync.dma_start(hn_d[:], tmp[:])

                    # ---- out partial = h2o_half @ h_new ----
                    nc.vector.tensor_copy(tt[:], tmp[:])
                    pe_mv(hov, tt, tmp2)
                    nc.sync.dma_start(o_d[:], tmp2[:])

    nc.compile()
    return nc


_CACHE = {}


def _tile32(vec):
    """flat (4096,) -> (128, 32): t[p, b] = vec[32 p + b]."""
    return np.ascontiguousarray(vec.reshape(128, NB).astype(np.float32))


def _tileA(M, scale, dt):
    """(4096, 4096) -> stationary-lhsT layout [p, (k, m, q)]:
    A[p, k, m, q] = M[32q + m, 32p + k]."""
    A = np.transpose((M * scale).reshape(128, NB, 128, NB), (2, 3, 1, 0))
    return np.ascontiguousarray(A).astype(dt).reshape(128, -1)


def _fingerprint(arrs):
    h = 0
    for a in arrs:
        a = np.asarray(a)
        h = hash((h, a.shape, a.dtype.str,
                  a.reshape(-1)[:8].tobytes(), a.reshape(-1)[-8:].tobytes(),
                  float(np.sum(a[..., ::257])) if a.size > 64 else
                  a.tobytes()))
    return h


def _get_runner(niters):
    key = f"runner{niters}"
    if key in _CACHE:
        return _CACHE[key]
    nc = _build(niters)
    bass2jax.install_neuronx_cc_hook()
    in_names, out_names, out_avals, zero_outs = [], [], [], []
    for alloc in nc.m.functions[0].allocations:
        if not isinstance(alloc, mybir.MemoryLocationSet):
            continue
        name = alloc.memorylocations[0].name
        if alloc.kind == "ExternalInput":
            in_names.append(name)
        elif alloc.kind == "ExternalOutput":
            out_names.append(name)
            shape = tuple(alloc.tensor_shape)
            dtype = mybir.dt.np(alloc.dtype)
            out_avals.append(jax.core.ShapedArray(shape, dtype))
            zero_outs.append(np.zeros(shape, dtype))
    partition_name = (nc.partition_id_tensor.name
                      if nc.partition_id_tensor else None)
    in_names = [n for n in in_names if n != partition_name]
    n_params = len(in_names)
    all_names = list(in_names) + list(out_names)
    if partition_name is not None:
        all_names.append(partition_name)
    donate = tuple(range(n_params, n_params + len(out_names)))

    def _body(*args):
        operands = list(args)
        if partition_name is not None:
            operands.append(bass2jax.partition_id_tensor())
        outs = bass2jax._bass_exec_p.bind(
            *operands,
            out_avals=tuple(out_avals),
            in_names=tuple(all_names),
            out_names=tuple(out_names),
            lowering_input_output_aliases=(),
            sim_require_finite=True,
            sim_require_nnan=True,
            nc=nc,
        )
        return tuple(outs)

    devices = jax.devices()[:2]
    mesh = bass2jax.Mesh(np.asarray(devices), ("core",))
    from jax.sharding import PartitionSpec, NamedSharding
    in_specs = (PartitionSpec("core"),) * (n_params + len(out_names))
    out_specs = (PartitionSpec("core"),) * len(out_names)
    fn = jax.jit(
        bass2jax.shard_map(_body, mesh=mesh, in_specs=in_specs,
                           out_specs=out_specs, check_rep=False),
        donate_argnums=donate, keep_unused=True)
    runner = dict(fn=fn, in_names=in_names, out_names=out_names,
                  zero_outs=zero_outs, mesh=mesh,
                  sharding=NamedSharding(mesh, PartitionSpec("core")))
    _CACHE[key] = runner
    return runner


_WEIGHT_NAMES = ("mq", "w1", "w2", "wx", "wh", "ho")


def _run(runner, in_maps):
    dev = _CACHE.setdefault("dev_weights", {})
    args = []
    for name in runner["in_names"]:
        glob = np.concatenate([in_maps[0][name], in_maps[1][name]], axis=0)
        if name in _WEIGHT_NAMES:
            fp = _CACHE.get("w_fp")
            cached = dev.get(name)
            if cached is None or cached[0] != fp:
                arr = jax.device_put(glob, runner["sharding"])
                arr.block_until_ready()
                dev[name] = (fp, arr)
            args.append(dev[name][1])
        else:
            args.append(glob)
    zeros = [np.zeros((2 * z.shape[0], *z.shape[1:]), z.dtype)
             for z in runner["zero_outs"]]
    out = runner["fn"](*args, *zeros)
    res = []
    for c in range(2):
        res.append({name: np.asarray(out[i]).reshape(
            2, *runner["zero_outs"][i].shape)[c]
            for i, name in enumerate(runner["out_names"])})
    return res


def kernel(x_f, x_b, h_f, h_b, t_f, t_b,
           i2h_W, i2h_b, h2o_W, h2o_b, f_W1, f_b1, f_W2, f_b2):
    args = [x_f, x_b, h_f, h_b, t_f, t_b, i2h_W, i2h_b, h2o_W, h2o_b,
            f_W1, f_b1, f_W2, f_b2]
    x_f, x_b, h_f, h_b, t_f, t_b, i2h_W, i2h_b, h2o_W, h2o_b, f_W1, f_b1, \
        f_W2, f_b2 = [np.asarray(a, np.float32) for a in args]

    wfp = _fingerprint([i2h_W, h2o_W, f_W1, f_W2, f_b1, f_b2, i2h_b])
    if _CACHE.get("w_fp") != wfp:
        G = (f_W1 @ f_W2).astype(np.float32)
        g0 = (f_W1 @ f_b2).astype(np.float32)
        bf = ml_dtypes.bfloat16
        _CACHE["w_prep"] = {
            "mq": _tileA(G, SM, ml_dtypes.float8_e3m4),
            "w1": _tileA(f_W1, 1.0, bf),
            "w2": _tileA(f_W2, 1.0, bf),
            "wx": _tileA(i2h_W[:, :NH], 1.0, bf),
            "wh": _tileA(i2h_W[:, NH:], 1.0, bf),
            "ho": [_tileA(h2o_W[:, :NH], 1.0, bf),
                   _tileA(h2o_W[:, NH:], 1.0, bf)],
            "g0": g0,
        }
        _CACHE["w_fp"] = wfp
        _CACHE.pop("dev_weights", None)
    wp = _CACHE["w_prep"]
    g0 = wp["g0"]

    in_maps = []
    for c, (x, h0, t) in enumerate([(x_f, h_f, t_f), (x_b, h_b, t_b)]):
        dt = float(t[1] - t[0])
        csts = np.zeros((128, NB, NCST), np.float32)
        csts[:, :, C_B1] = _tile32(f_b1)
        csts[:, :, C_CG0A] = _tile32(dt / 2.0 * g0)
        csts[:, :, C_CG0B] = _tile32(dt * g0)
        csts[:, :, C_H0SD] = _tile32(h0 + NSTEP * dt * f_b2)
        csts[:, :, C_BG] = _tile32(i2h_b)
        scal = np.zeros((128, 4), np.float32)
        scal[:, S_CA2] = dt / 2.0 / SMT
        scal[:, S_CB2] = dt / SMT
        scal[:, S_CD6] = dt / 6.0 / SMT
        scal[:, S_CD] = dt / 6.0
        in_maps.append({
            "mq": wp["mq"], "w1": wp["w1"], "w2": wp["w2"],
            "wx": wp["wx"], "wh": wp["wh"], "ho": wp["ho"][c],
            "h0c": _tile32(h0).astype(ml_dtypes.bfloat16),
            "xc": _tile32(x.reshape(-1)).astype(ml_dtypes.bfloat16),
            "csts": csts.reshape(128, -1),
            "scal": scal,
        })

    runner = _get_runner(int(_CACHE.get("niters", 1)))
    res = _run(runner, in_maps)
    _CACHE["last_results"] = res

    hf = res[0]["hn"].reshape(-1)
    hb = res[1]["hn"].reshape(-1)
    out = (res[0]["o_part"].reshape(-1) +
           res[1]["o_part"].reshape(-1) + h2o_b)
    return out, hf, hb


# revision 4
# speedup vs baseline: 1.0683x; 1.0683x over previous
"""Trainium2 Bass kernel for the bidirectional GRU-ODE (nn_CODEBiGRU).

All-TensorEngine design (collective-free, 2 cores, one chain per core):
  - Host folds G = W1 @ W2, g0 = W1 @ b2.  RK4 runs in pre-activation
    space: d_q = G t_q, u_{q+1} = u1 + c_q d_q; h-updates telescope into
    h_T = (h0 + 15 dt b2) + dt/6 W2 (sum_s T_s).
  - Everything is column-tiled: cols[p, j] = flat[32p + j].  All matvecs
    use stationary-weight matmuls: lhsT(k,m)[p,q] = W[32q+m, 32p+k],
    out cols[q, m] accumulated over k in PSUM, m-outer/k-inner
    (sequential accumulation groups; interleaved groups are numerically
    broken on this HW).
  - The 61 sequential G-matvecs use SBUF-resident fp8-e3m4 G (scaled
    x64, t scaled x8, 1/512 folded into step constants): 1024 matmuls /
    37us per eval.  Measured end-to-end rel-err ~9e-3 (gate 2e-2).
  - Init + finale (W1, W2, Wx, Wh x2, h2o-half) stream bf16 weights in
    the same layout through double-buffered 4MB chunks on both DGE
    queues (SP + Activation), 1024 matmuls each, 8 PSUM partial columns
    reduced on DVE.  The i2h x-part is computed once and reused for both
    GRU gates.
  - Whole iteration sits in For_i(0, niters) for delta-timing.
  - Host: weight re-layouts cached by fingerprint; per-core weight
    tensors are device_put once and reused across calls (axon tunnel is
    ~40MB/s, so re-upload would dominate).
"""
import sys
import numpy as np

sys.path.insert(0, "/opt/trn_rl_repo")

import ml_dtypes  # noqa: E402
import jax  # noqa: E402
import concourse.bass as bass  # noqa: E402
import concourse.tile as tile  # noqa: E402
from concourse import bacc, mybir, bass2jax  # noqa: E402

NH = 4096
NB = 32                 # col blocks (tiled [128, 32], flat = 32p + b)
NSTEP = 15
F32 = mybir.dt.float32
BF16 = mybir.dt.bfloat16
F8E3 = mybir.dt.float8e3
AF = mybir.ActivationFunctionType
ALU = mybir.AluOpType

SM = 64.0               # fp8 scale on G
STQ = 8.0               # fp8 scale on t
SMT = SM * STQ

KBSZ = 1                # streamed slab-group size (k-tiles per chunk)
NKB = NB // KBSZ        # chunks per streamed matvec
NREG = 6                # stream regions (DMA lookahead = NREG - 1 chunks)

C_B1, C_CG0A, C_CG0B, C_H0SD, C_BG = range(5)
NCST = 5
S_CA2, S_CB2, S_CD6, S_CD = range(4)


def _build(niters=1):
    nc = bacc.Bacc("TRN2", target_bir_lowering=False, debug=False,
                   num_devices=2)

    mq_d = nc.dram_tensor("mq", [128, NB * NB * 128], F8E3, kind="ExternalInput")
    w1_d = nc.dram_tensor("w1", [128, NB * NB * 128], BF16, kind="ExternalInput")
    w2_d = nc.dram_tensor("w2", [128, NB * NB * 128], BF16, kind="ExternalInput")
    wx_d = nc.dram_tensor("wx", [128, NB * NB * 128], BF16, kind="ExternalInput")
    wh_d = nc.dram_tensor("wh", [128, NB * NB * 128], BF16, kind="ExternalInput")
    ho_d = nc.dram_tensor("ho", [128, NB * NB * 128], BF16, kind="ExternalInput")
    h0c_d = nc.dram_tensor("h0c", [128, NB], BF16, kind="ExternalInput")
    xc_d = nc.dram_tensor("xc", [128, NB], BF16, kind="ExternalInput")
    cst_d = nc.dram_tensor("csts", [128, NB * NCST], F32, kind="ExternalInput")
    scal_d = nc.dram_tensor("scal", [128, 4], F32, kind="ExternalInput")

    o_d = nc.dram_tensor("o_part", [128, NB], F32, kind="ExternalOutput")
    hn_d = nc.dram_tensor("hn", [128, NB], F32, kind="ExternalOutput")

    def aview(d):
        return d[:].rearrange("p (k m q) -> p k m q", k=NB, m=NB)

    mqv, w1v, w2v, wxv, whv, hov = map(
        aview, (mq_d, w1_d, w2_d, wx_d, wh_d, ho_d))

    with tile.TileContext(nc) as tc:
        with tc.tile_pool(name="base", bufs=1) as base, \
             tc.tile_pool(name="psum", bufs=1, space="PSUM") as pp:

            ma = base.tile([128, NB, NB, 128], F8E3, tag="ma")       # 128KB/p
            sregs = [base.tile([128, KBSZ, NB, 128], BF16,
                               tag=f"sreg{i}", name=f"sreg{i}")
                     for i in range(NREG)]                           # 16KB/p ea

            u1 = base.tile([128, NB], F32, tag="u1")
            ux = base.tile([128, NB], F32, tag="ux")
            sA = base.tile([128, NB], F32, tag="sA")
            sB = base.tile([128, NB], F32, tag="sB")
            dsum = base.tile([128, NB], F32, tag="dsum")
            tsum = base.tile([128, NB], F32, tag="tsum")
            tbf = base.tile([128, NB], BF16, tag="tbf")
            tq = base.tile([128, NB], F8E3, tag="tq")
            tt = base.tile([128, NB], BF16, tag="tt")
            h0c = base.tile([128, NB], BF16, tag="h0c")
            xc = base.tile([128, NB], BF16, tag="xc")
            px = base.tile([128, NB], F32, tag="px")
            tmp = base.tile([128, NB], F32, tag="tmp")
            tmp2 = base.tile([128, NB], F32, tag="tmp2")
            hfin = base.tile([128, NB], F32, tag="hfin")
            gg = base.tile([128, NB], F32, tag="gg")
            csts = base.tile([128, NB, NCST], F32, tag="csts")
            scal = base.tile([128, 4], F32, tag="scal")
            ps = pp.tile([128, NB], F32, tag="ps")
            ps2 = pp.tile([128, NB, NKB], F32, tag="ps2")

            qeng = [nc.sync, nc.scalar]

            def pe_mv(wview, rhs_cols, acc):
                """acc[q, m] = sum_k W(k,m).T @ rhs[:, k], streamed bf16
                weights, NKB psum partials reduced on DVE."""
                def ld(kb, qi):
                    qeng[qi % 2].dma_start(
                        sregs[kb % NREG][:].rearrange("p a m q -> p (a m q)"),
                        wview[:, kb * KBSZ:(kb + 1) * KBSZ, :, :].rearrange(
                            "p a m q -> p (a m q)"))
                for j in range(min(NREG - 1, NKB)):
                    ld(j, j)
                for kb in range(NKB):
                    if kb + NREG - 1 < NKB:
                        ld(kb + NREG - 1, kb + NREG - 1)
                    reg = sregs[kb % NREG]
                    for m in range(NB):
                        for kk in range(KBSZ):
                            nc.tensor.matmul(
                                ps2[:, m, kb:kb + 1],
                                reg[:, kk, m, :],
                                rhs_cols[:, kb * KBSZ + kk:kb * KBSZ + kk + 1],
                                start=(kk == 0), stop=(kk == KBSZ - 1))
                nc.vector.tensor_reduce(acc[:], ps2[:], mybir.AxisListType.X,
                                        ALU.add)

            def mm_eval():
                """ps[:, m] = sum_k MA(k,m).T @ tq[:, k]  (= SMT * G @ t)."""
                for m in range(NB):
                    for k in range(NB):
                        nc.tensor.matmul(
                            ps[:, m:m + 1],
                            ma[:, k, m, :],
                            tq[:, k:k + 1],
                            start=(k == 0), stop=(k == NB - 1))

            def stage_tail(u_next_like):
                nc.scalar.activation(tbf[:], u_next_like[:], AF.Tanh)
                nc.vector.tensor_scalar_mul(tq[:], tbf[:], STQ)

            with tc.For_i(0, niters) as _it:
                nc.sync.dma_start(csts[:].rearrange("p a c -> p (a c)"),
                                  cst_d[:])
                nc.sync.dma_start(scal[:], scal_d[:])
                nc.sync.dma_start(h0c[:], h0c_d[:])
                nc.sync.dma_start(xc[:], xc_d[:])
                capA2 = scal[:, S_CA2:S_CA2 + 1]
                capB2 = scal[:, S_CB2:S_CB2 + 1]
                capD6 = scal[:, S_CD6:S_CD6 + 1]
                capD = scal[:, S_CD:S_CD + 1]

                # ---- u1 = W1 @ h0 + b1 (streamed PE matvec) ----
                pe_mv(w1v, h0c, u1)
                nc.vector.tensor_add(u1[:], u1[:], csts[:, :, C_B1])

                # ---- Px = Wx @ x + ib (ODE-independent, done up front) ----
                pe_mv(wxv, xc, px)
                nc.vector.tensor_add(px[:], px[:], csts[:, :, C_BG])

                # load G (fp8, 16MB) in 8 chunks, alternating DGE queues
                for i in range(8):
                    qeng[i % 2].dma_start(
                        ma[:, i * 4:(i + 1) * 4, :, :].rearrange(
                            "p a m q -> p (a m q)"),
                        mqv[:, i * 4:(i + 1) * 4, :, :].rearrange(
                            "p a m q -> p (a m q)"))

                nc.scalar.activation(tbf[:], u1[:], AF.Tanh)
                nc.vector.tensor_copy(tsum[:], tbf[:])
                nc.vector.tensor_scalar_mul(tq[:], tbf[:], STQ)

                # ---- RK4: 15 steps x 4 PE-matvecs ----
                with tc.For_i(0, NSTEP) as _s:
                    nc.vector.tensor_add(sA[:], u1[:], csts[:, :, C_CG0A])
                    nc.vector.tensor_add(sB[:], u1[:], csts[:, :, C_CG0B])
                    # q1
                    mm_eval()
                    nc.vector.tensor_scalar_mul(dsum[:], ps[:], 1.0)
                    nc.vector.scalar_tensor_tensor(
                        ux[:], ps[:], capA2, sA[:], ALU.mult, ALU.add)
                    stage_tail(ux)
                    nc.vector.scalar_tensor_tensor(
                        tsum[:], tbf[:], 2.0, tsum[:], ALU.mult, ALU.add)
                    # q2
                    mm_eval()
                    nc.vector.scalar_tensor_tensor(
                        dsum[:], ps[:], 2.0, dsum[:], ALU.mult, ALU.add)
                    nc.vector.scalar_tensor_tensor(
                        ux[:], ps[:], capA2, sA[:], ALU.mult, ALU.add)
                    stage_tail(ux)
                    nc.vector.scalar_tensor_tensor(
                        tsum[:], tbf[:], 2.0, tsum[:], ALU.mult, ALU.add)
                    # q3
                    mm_eval()
                    nc.vector.scalar_tensor_tensor(
                        dsum[:], ps[:], 2.0, dsum[:], ALU.mult, ALU.add)
                    nc.vector.scalar_tensor_tensor(
                        ux[:], ps[:], capB2, sB[:], ALU.mult, ALU.add)
                    stage_tail(ux)
                    nc.vector.tensor_add(tsum[:], tsum[:], tbf[:])
                    # q4
                    mm_eval()
                    nc.vector.tensor_add(dsum[:], dsum[:], ps[:])
                    nc.vector.scalar_tensor_tensor(
                        u1[:], dsum[:], capD6, u1[:], ALU.mult, ALU.add)
                    nc.vector.tensor_add(u1[:], u1[:], csts[:, :, C_CG0B])
                    stage_tail(u1)
                    nc.vector.tensor_add(tsum[:], tsum[:], tbf[:])

                # tsum overcounts tanh(u1_15): subtract
                nc.vector.tensor_sub(tsum[:], tsum[:], tbf[:])

                if True:
                    # ---- h_T = (h0 + 15 dt b2) + dt/6 W2 tsum ----
                    nc.vector.tensor_copy(tt[:], tsum[:])
                    pe_mv(w2v, tt, tmp)
                    nc.vector.scalar_tensor_tensor(
                        hfin[:], tmp[:], capD, csts[:, :, C_H0SD],
                        ALU.mult, ALU.add)

                    # ---- GRU (chain-local, all in col space) ----
                    nc.vector.tensor_copy(tt[:], hfin[:])
                    pe_mv(whv, tt, tmp)
                    nc.vector.tensor_add(gg[:], px[:], tmp[:])
                    nc.scalar.activation(gg[:], gg[:], AF.Sigmoid)
                    nc.vector.tensor_mul(tt[:], gg[:], hfin[:])
                    pe_mv(whv, tt, tmp)
                    nc.vector.tensor_add(tmp[:], px[:], tmp[:])
                    nc.scalar.activation(tmp[:], tmp[:], AF.Tanh)
                    # h_new = h_hat + g*(h - h_hat)
                    nc.vector.tensor_sub(tmp2[:], hfin[:], tmp[:])
                    nc.vector.tensor_mul(tmp2[:], gg[:], tmp2[:])
                    nc.vector.tensor_add(tmp[:], tmp[:], tmp2[:])
                    nc.sync.dma_start(hn_d[:], tmp[:])

                    # ---- out partial = h2o_half @ h_new ----
                    nc.vector.tensor_copy(tt[:], tmp[:])
                    pe_mv(hov, tt, tmp2)
                    nc.sync.dma_start(o_d[:], tmp2[:])

    nc.compile()
    return nc


_CACHE = {}


def _tile32(vec):
    """flat (4096,) -> (128, 32): t[p, b] = vec[32 p + b]."""
    return np.ascontiguousarray(vec.reshape(128, NB).astype(np.float32))


def _tileA(M, scale, dt):
    """(4096, 4096) -> stationary-lhsT layout [p, (k, m, q)]:
    A[p, k, m, q] = M[32q + m, 32p + k]."""
    A = np.transpose((M * scale).reshape(128, NB, 128, NB), (2, 3, 1, 0))
    return np.ascontiguousarray(A).astype(dt).reshape(128, -1)


def _fingerprint(arrs):
    h = 0
    for a in arrs:
        a = np.asarray(a)
        h = hash((h, a.shape, a.dtype.str,
                  a.reshape(-1)[:8].tobytes(), a.reshape(-1)[-8:].tobytes(),
                  float(np.sum(a[..., ::257])) if a.size > 64 else
                  a.tobytes()))
    return h


def _get_runner(niters):
    key = f"runner{niters}"
    if key in _CACHE:
        return _CACHE[key]
    nc = _build(niters)
    bass2jax.install_neuronx_cc_hook()
    in_names, out_names, out_avals, zero_outs = [], [], [], []
    for alloc in nc.m.functions[0].allocations:
        if not isinstance(alloc, mybir.MemoryLocationSet):
            continue
        name = alloc.memorylocations[0].name
        if alloc.kind == "ExternalInput":
            in_names.append(name)
        elif alloc.kind == "ExternalOutput":
            out_names.append(name)
            shape = tuple(alloc.tensor_shape)
            dtype = mybir.dt.np(alloc.dtype)
            out_avals.append(jax.core.ShapedArray(shape, dtype))
            zero_outs.append(np.zeros(shape, dtype))
    partition_name = (nc.partition_id_tensor.name
                      if nc.partition_id_tensor else None)
    in_names = [n for n in in_names if n != partition_name]
    n_params = len(in_names)
    all_names = list(in_names) + list(out_names)
    if partition_name is not None:
        all_names.append(partition_name)
    donate = tuple(range(n_params, n_params + len(out_names)))

    def _body(*args):
        operands = list(args)
        if partition_name is not None:
            operands.append(bass2jax.partition_id_tensor())
        outs = bass2jax._bass_exec_p.bind(
            *operands,
            out_avals=tuple(out_avals),
            in_names=tuple(all_names),
            out_names=tuple(out_names),
            lowering_input_output_aliases=(),
            sim_require_finite=True,
            sim_require_nnan=True,
            nc=nc,
        )
        return tuple(outs)

    devices = jax.devices()[:2]
    mesh = bass2jax.Mesh(np.asarray(devices), ("core",))
    from jax.sharding import PartitionSpec, NamedSharding
    in_specs = (PartitionSpec("core"),) * (n_params + len(out_names))
    out_specs = (PartitionSpec("core"),) * len(out_names)
    fn = jax.jit(
        bass2jax.shard_map(_body, mesh=mesh, in_specs=in_specs,
                           out_specs=out_specs, check_rep=False),
        donate_argnums=donate, keep_unused=True)
    runner = dict(fn=fn, in_names=in_names, out_names=out_names,
                  zero_outs=zero_outs, mesh=mesh,
                  sharding=NamedSharding(mesh, PartitionSpec("core")))
    _CACHE[key] = runner
    return runner


_WEIGHT_NAMES = ("mq", "w1", "w2", "wx", "wh", "ho")


def _run(runner, in_maps):
    dev = _CACHE.setdefault("dev_weights", {})
    args = []
    for name in runner["in_names"]:
        glob = np.concatenate([in_maps[0][name], in_maps[1][name]], axis=0)
        if name in _WEIGHT_NAMES:
            fp = _CACHE.get("w_fp")
            cached = dev.get(name)
            if cached is None or cached[0] != fp:
                arr = jax.device_put(glob, runner["sharding"])
                arr.block_until_ready()
                dev[name] = (fp, arr)
            args.append(dev[name][1])
        else:
            args.append(glob)
    zeros = [np.zeros((2 * z.shape[0], *z.shape[1:]), z.dtype)
             for z in runner["zero_outs"]]
    out = runner["fn"](*args, *zeros)
    res = []
    for c in range(2):
        res.append({name: np.asarray(out[i]).reshape(
            2, *runner["zero_outs"][i].shape)[c]
            for i, name in enumerate(runner["out_names"])})
    return res


def kernel(x_f, x_b, h_f, h_b, t_f, t_b,
           i2h_W, i2h_b, h2o_W, h2o_b, f_W1, f_b1, f_W2, f_b2):
    args = [x_f, x_b, h_f, h_b, t_f, t_b, i2h_W, i2h_b, h2o_W, h2o_b,
            f_W1, f_b1, f_W2, f_b2]
    x_f, x_b, h_f, h_b, t_f, t_b, i2h_W, i2h_b, h2o_W, h2o_b, f_W1, f_b1, \
        f_W2, f_b2 = [np.asarray(a, np.float32) for a in args]

    wfp = _fingerprint([i2h_W, h2o_W, f_W1, f_W2, f_b1, f_b2, i2h_b])
    if _CACHE.get("w_fp") != wfp:
        G = (f_W1 @ f_W2).astype(np.float32)
        g0 = (f_W1 @ f_b2).astype(np.float32)
        bf = ml_dtypes.bfloat16
        _CACHE["w_prep"] = {
            "mq": _tileA(G, SM, ml_dtypes.float8_e3m4),
            "w1": _tileA(f_W1, 1.0, bf),
            "w2": _tileA(f_W2, 1.0, bf),
            "wx": _tileA(i2h_W[:, :NH], 1.0, bf),
            "wh": _tileA(i2h_W[:, NH:], 1.0, bf),
            "ho": [_tileA(h2o_W[:, :NH], 1.0, bf),
                   _tileA(h2o_W[:, NH:], 1.0, bf)],
            "g0": g0,
        }
        _CACHE["w_fp"] = wfp
        _CACHE.pop("dev_weights", None)
    wp = _CACHE["w_prep"]
    g0 = wp["g0"]

    in_maps = []
    for c, (x, h0, t) in enumerate([(x_f, h_f, t_f), (x_b, h_b, t_b)]):
        dt = float(t[1] - t[0])
        csts = np.zeros((128, NB, NCST), np.float32)
        csts[:, :, C_B1] = _tile32(f_b1)
        csts[:, :, C_CG0A] = _tile32(dt / 2.0 * g0)
        csts[:, :, C_CG0B] = _tile32(dt * g0)
        csts[:, :, C_H0SD] = _tile32(h0 + NSTEP * dt * f_b2)
        csts[:, :, C_BG] = _tile32(i2h_b)
        scal = np.zeros((128, 4), np.float32)
        scal[:, S_CA2] = dt / 2.0 / SMT
        scal[:, S_CB2] = dt / SMT
        scal[:, S_CD6] = dt / 6.0 / SMT
        scal[:, S_CD] = dt / 6.0
        in_maps.append({
            "mq": wp["mq"], "w1": wp["w1"], "w2": wp["w2"],
            "wx": wp["wx"], "wh": wp["wh"], "ho": wp["ho"][c],
            "h0c": _tile32(h0).astype(ml_dtypes.bfloat16),
            "xc": _tile32(x.reshape(-1)).astype(ml_dtypes.bfloat16),
            "csts": csts.reshape(128, -1),
            "scal": scal,
        })

    runner = _get_runner(int(_CACHE.get("niters", 1)))
    res = _run(runner, in_maps)
    _CACHE["last_results"] = res

    hf = res[0]["hn"].reshape(-1)
    hb = res[1]["hn"].reshape(-1)
    out = (res[0]["o_part"].reshape(-1) +
           res[1]["o_part"].reshape(-1) + h2o_b)
    return out, hf, hb
